# revision 14
# baseline (speedup 1.0000x reference)
"""Binarized-CNN BasicBlock (2x conv3x3 256ch + train-mode BN + hardtanh +
residual) on 8 trn2 NeuronCores, data-parallel over the batch.

Key structure:
  - binarize(x) in {-1,+1} stored as fp8 -> conv = exact integer sums in
    fp32 PSUM via 18 accumulating DoubleRow matmuls (9 taps x 2 out-halves)
    over a zero-padded 30-row x 29-col spatial layout (28 data cols + ONE
    shared zero pad column per row: tap reads of col -1 / col 28 both land
    on a neighbouring row's pad byte, so the matmul free dim is 14x29=406
    instead of 15x30=450 -> ~10% fewer PE cycles).
  - conv bias b1/b2 cancel under training-mode BN (shift invariance) and are
    never applied.
  - sign(hardtanh(bn(v))) == sign(v*scale + bias) so conv2's input needs only
    an affine threshold of conv1's raw output.
  - BN statistics: one DVE bn_stats per drained slab (count/mean/M2 in a
    single pass) + one bn_aggr per half -> no drain accums, no Square passes,
    no activation-table churn. (scalar_tensor_tensor / tensor_tensor_reduce
    crash trn2 hardware in this vintage - bisected; bn_stats is native.)
  - conv2: DVE adds the re-DMAed bf16 residual into PSUM, ScalarE drains
    t2 -> v.
  - Per-core stats are AllReduced (tiny, latency-bound ~8us); a dummy warmup
    collective at kernel start absorbs the first collective's ~11us stream
    warmup. For the LAST group of each stat phase the drains go to ScalarE so
    the DVE closes bn_stats/bn_aggr ~3us after the last matmul.
  - y is stored bf16 (halves the output DMA) and upcast to f32 on the host;
    the values are hardtanh-clipped to [-1,1] so bf16 costs <0.4% abs error.
  - Fully pipelined: conv1 starts while input chunks still stream in; conv2's
    plane-0 sign threshold passes hide under conv1's second half; the first
    half's output pass hides under the second half's conv; everything after
    the last matmul is just stats-AR + the 16-image second-half output pass.
"""

import os
import numpy as np
import ml_dtypes

import concourse.bacc as bacc
import concourse.tile as tile
from concourse import mybir
from concourse.bass_utils import run_bass_kernel_spmd

# ---------------- problem constants (hardcoded) ----------------
N_CORES = 8
N_FULL, C, H, W = 128, 256, 28, 28
NPC = N_FULL // N_CORES          # 16 images per core
WPAD = 29                        # 28 data cols + 1 shared zero pad col
IMG = 30 * WPAD                  # 870 bytes per padded plane (rows 0/29 pad)
MARG = 32                        # margin so shifted reads stay in-bounds
IMGB = 880                       # plane stride inside an image pair (55*16)
IMGP = 2 * IMGB                  # bytes per image (both planes)
BUF = MARG + (NPC + 1) * IMGP    # + one image of tail slack for AP views
SLAB = 406                       # matmul moving free dim (14 rows x 29)
NSLABS = NPC * 2                 # 32 (2 slabs per image)
GROUP = 2                        # psum tiles per accumulation group: small
                                 # groups keep 4 in flight in the 8 PSUM
                                 # banks, so drain lag never idles the PE
NGROUPS = NSLABS // GROUP        # 16 (one group == one image)
NTOT = N_FULL * H * W            # 100352 samples per channel (full batch)
EPS = 1e-5
QI = 2                           # images per streaming chunk

BF16 = mybir.dt.bfloat16
FP8 = mybir.dt.float8e4
F32 = mybir.dt.float32
AF = mybir.ActivationFunctionType
ALU = mybir.AluOpType
NP_FP8 = mybir.dt.np(FP8)

# hardware-bisect flags (defaults = final config)
F_WARMUP = os.environ.get("K_WARMUP", "1") == "1"
F_NEWOPS = os.environ.get("K_NEWOPS", "1") == "1"
F_PADMEMSET = os.environ.get("K_PADMEMSET", "1") == "1"
F_TLPRE = os.environ.get("K_TLPRE", "1") == "1"
F_YBF16 = os.environ.get("K_YBF16", "1") == "1"
YDT_NP = __import__("ml_dtypes").bfloat16 if F_YBF16 else np.float32



def _interior(buf, ci, a, q):
    """[128, q, 28, 28] view of the valid pixels of plane ci of images
    a..a+q in a plane-interleaved padded [128, BUF] buffer."""
    base = MARG + ci * IMGB + a * IMGP + WPAD
    v = buf[:, base: base + q * IMGP]
    v = v.rearrange("p (n b) -> p n b", b=IMGP)[:, :, : 28 * WPAD]
    v = v.rearrange("p n (r c) -> p n r c", r=28, c=WPAD)
    return v[:, :, :, :28]


def _rhs(buf, s, off, doff):
    """DoubleRow rhs AP [128, 2, SLAB] for slab s shifted by tap offset."""
    n_img, h = divmod(s, 2)
    base = MARG + n_img * IMGP + h * SLAB + doff[off]
    v = buf[:, base: base + IMGP]
    return v.rearrange("p (k b) -> p k b", k=2)[:, :, :SLAB]


def _x_dram_ap(xd, a, q, mi):
    """DRAM AP for images a..a+q, channel tile mi -> [128, q, 784]."""
    sl = xd[a: a + q, mi * 128: (mi + 1) * 128, :, :]
    return sl.rearrange("n c h w -> c n (h w)")


def _pad_memsets(nc, buf):
    """Zero only the pad bytes of a padded sign buffer: margin, tail slack,
    top/bottom pad rows, the shared pad column, inter-plane slack."""
    if not F_PADMEMSET:
        nc.vector.memset(buf[:].bitcast(F32), 0.0)
        return
    nc.vector.memset(buf[:, :MARG], 0.0)
    nc.vector.memset(buf[:, MARG + NPC * IMGP:].bitcast(F32), 0.0)
    planes = buf[:, MARG: MARG + NPC * IMGP].rearrange(
        "p (n b) -> p n b", b=IMGB)
    nc.vector.memset(planes[:, :, :WPAD], 0.0)            # top pad rows
    nc.vector.memset(planes[:, :, 29 * WPAD:], 0.0)       # bottom row+slack
    body = planes[:, :, WPAD: 29 * WPAD].rearrange(
        "p n (r c) -> p n r c", r=28, c=WPAD)
    nc.vector.memset(body[:, :, :, 28:], 0.0)             # shared pad col


def _build():
    nc = bacc.Bacc(
        "TRN2",
        target_bir_lowering=False,
        debug=False,
        num_devices=N_CORES,
    )
    xd = nc.dram_tensor("x", [NPC, C, H, W], BF16, kind="ExternalInput")
    # DoubleRow lhsT layout: [ki=128, off=9, ko=2, o=256] fp8, channel = ko*128+ki
    w1d = nc.dram_tensor("w1s", [128, 9, 2, 256], FP8, kind="ExternalInput")
    w2d = nc.dram_tensor("w2s", [128, 9, 2, 256], FP8, kind="ExternalInput")
    bnd = nc.dram_tensor("bnp", [128, 8], F32, kind="ExternalInput")
    yd = nc.dram_tensor("y", [NPC, C, H, W], BF16 if F_YBF16 else F32,
                        kind="ExternalOutput")

    # tap offsets in the padded layout
    doff = [dy * WPAD + (dx - 1) for dy in range(3) for dx in range(3)]

    with tile.TileContext(nc) as tc:
        with (
            tc.tile_pool(name="wp", bufs=1) as wp,
            tc.tile_pool(name="xsp", bufs=1) as xsp,
            tc.tile_pool(name="vp", bufs=1) as vp,
            tc.tile_pool(name="small", bufs=1) as small,
            tc.tile_pool(name="instage", bufs=6) as instage,
            tc.tile_pool(name="rstage", bufs=4) as rstage,
            tc.tile_pool(name="upool", bufs=4) as upool,
            tc.tile_pool(name="psum", bufs=8, space="PSUM") as psum,
            tc.tile_pool(name="dram", bufs=1, space="DRAM") as dram,
        ):
            wsb = {
                conv: wp.tile([128, 9, 2, 256], FP8, tag=f"w{conv}",
                              name=f"w{conv}")
                for conv in (1, 2)
            }
            bnp = small.tile([128, 8], F32, tag="bnp")
            eps_sb = small.tile([128, 1], F32, tag="eps")
            nc.vector.memset(eps_sb[:], EPS)
            # preload the ScalarE Sign table before any input data arrives so
            # the first sign pass doesn't pay the 1.3us table load
            if F_TLPRE:
                tl = small.tile([128, 1], F32, tag="tl")
                nc.scalar.activation(out=tl[:], in_=eps_sb[:], func=AF.Sign)

            # dummy warmup collective: the first collective of a NEFF pays a
            # ~11us stream-warmup trigger delay; burn it here where it
            # overlaps the input DMA instead of in conv1's stats.
            if F_WARMUP:
                warm = small.tile([128, 1], F32, tag="warm")
                nc.vector.memset(warm[:], 0.0)
                warm_in = dram.tile([128, 1], F32, tag="warmin")
                warm_out = dram.tile([128, 1], F32, tag="warmout")
                nc.gpsimd.dma_start(out=warm_in[:], in_=warm[:])
                nc.gpsimd.collective_compute(
                    "AllReduce",
                    ALU.add,
                    replica_groups=[list(range(N_CORES))],
                    ins=[warm_in.opt()],
                    outs=[warm_out.opt()],
                )

            # ---- sign-input buffers: zero only the pad bytes ----
            xs_all = xsp.tile([128, BUF], FP8, tag="xs")
            xs2_all = xsp.tile([128, BUF], FP8, tag="xs2")
            _pad_memsets(nc, xs_all)
            _pad_memsets(nc, xs2_all)
            # conv outputs stored COMPACT (valid pixels only, [128, 16*784]):
            # v[m] holds conv1's raw output, later overwritten with
            # t2 = conv2 + residual during conv2's drains.
            v = [
                vp.tile([128, NPC * 784], BF16, name=f"v{ci}", tag=f"v{ci}")
                for ci in range(2)
            ]
            # ---- load x, binarize into padded layout; first chunks are a
            # single image so conv1 starts as early as possible. The weights
            # ride the gpsimd SWDGE queue so they never serialize behind the
            # x stream on the HWDGE ring. ----
            nc.gpsimd.dma_start(out=wsb[1][:], in_=w1d[:])
            chunks = [(0, 1), (1, 1)] + [(a, QI) for a in range(2, NPC, QI)]
            for a, q in chunks:
                for ci in range(2):
                    st = instage.tile([128, QI * 784], BF16, tag="xin")
                    nc.sync.dma_start(
                        out=st[:, : q * 784].rearrange(
                            "p (n f) -> p n f", n=q),
                        in_=_x_dram_ap(xd, a, q, ci),
                    )
                    nc.scalar.activation(
                        out=_interior(xs_all, ci, a, q),
                        in_=st[:, : q * 784].rearrange(
                            "p (n r c) -> p n r c", r=28, c=28),
                        func=AF.Sign,
                    )
            # w2 is first needed at conv2, bnp at the first bn_coeffs
            nc.gpsimd.dma_start(out=bnp[:], in_=bnd[:])
            nc.gpsimd.dma_start(out=wsb[2][:], in_=w2d[:])

            # ---- per-(conv, half) per-slab bn_stats tiles: 6 cols/slab ----
            bst = {}
            for layer in (1, 2):
                for m in range(2):
                    bst[layer, m] = small.tile(
                        [128, NSLABS * 6], F32, name=f"bst{layer}{m}",
                        tag=f"bst{layer}{m}")

            def conv_half(idx, src_all, m, inject=None):
                """One output-channel half of a 3x3 conv. dst = v[m].
                Per slab, one DVE bn_stats over the drained (compact) v slab
                gives count/mean/M2 -> no drain accums, no square passes.
                idx==1: drains on DVE (ScalarE keeps only the DMA-decoupled
                sign stream, so a late collective can never stall the PE via
                the ScalarE FIFO). idx==2: the DVE adds the re-DMAed residual
                into PSUM, ScalarE drains t2 -> v. inject(g) adds ops after
                group g."""
                bstv = bst[idx, m]
                rst = None
                for g in range(NGROUPS):
                    last = g == NGROUPS - 1
                    if idx == 2 and g % QI == 0:
                        rst = rstage.tile(
                            [128, QI * 784], BF16, name=f"rst{m}_{g}",
                            tag="rst")
                        nc.sync.dma_start(
                            out=rst.rearrange("p (n f) -> p n f", n=QI),
                            in_=_x_dram_ap(xd, g, QI, m),
                        )
                    ps = [
                        psum.tile([128, SLAB], F32, name=f"c{idx}ps{m}_{g}_{i}",
                                  tag="ps")
                        for i in range(GROUP)
                    ]
                    for off in range(9):
                        lhsT = wsb[idx][:, off, :, m * 128: m * 128 + 128]
                        for s4 in range(GROUP):
                            s = g * GROUP + s4
                            nc.tensor.matmul(
                                ps[s4][:],
                                lhsT,
                                _rhs(src_all, s, off, doff),
                                start=(off == 0),
                                stop=(off == 8),
                                perf_mode=mybir.MatmulPerfMode.DoubleRow,
                            )
                    for s4 in range(GROUP):
                        s = g * GROUP + s4
                        n_img, h = divmod(s, 2)
                        # slab covers out rows [14h, 14h+14) of image n_img;
                        # cols 0..27 of each 29-wide psum row are valid
                        src_ap = ps[s4].rearrange(
                            "p (r c) -> p r c", r=14, c=WPAD)[:, :, :28]
                        cb = n_img * 784 + h * 392
                        dst_ap = v[m][:, cb: cb + 392].rearrange(
                            "p (r c) -> p r c", r=14, c=28)
                        if idx == 2:
                            rb = (n_img - (g & ~(QI - 1))) * 784 + h * 392
                            res_ap = rst[:, rb: rb + 392].rearrange(
                                "p (r c) -> p r c", r=14, c=28)
                            nc.vector.tensor_tensor(
                                out=src_ap, in0=src_ap, in1=res_ap,
                                op=ALU.add,
                            )
                            nc.scalar.activation(
                                out=dst_ap, in_=src_ap, func=AF.Identity,
                            )
                        elif last:
                            nc.scalar.activation(
                                out=dst_ap, in_=src_ap, func=AF.Identity,
                            )
                        else:
                            nc.vector.tensor_copy(out=dst_ap, in_=src_ap)
                        # one-pass mean/M2 over the compact bf16 slab
                        nc.vector.bn_stats(
                            out=bstv[:, s * 6: s * 6 + 6],
                            in_=v[m][:, cb: cb + 392],
                        )
                    if inject is not None:
                        inject(g)

            def start_allreduce(layer, m, after=None):
                """Aggregate this half's per-slab stats into (sum, sumsq) and
                kick off its AllReduce. `after` (the previous AR's result
                tile) is folded in as a zero so the scheduler cannot hoist
                this AR's long-waiting stat DMA ahead of that result fetch in
                the engine FIFO (head-of-line blocking)."""
                npts = float(NPC * 784)
                stat = small.tile(
                    [128, 2], F32, name=f"st{layer}{m}", tag=f"stat{layer}{m}")
                if after is not None:
                    nc.vector.tensor_scalar_mul(stat[:, 0:1], after[:, 0:1], 0.0)
                agg = small.tile([128, 2], F32, tag=f"agg{layer}{m}")
                nc.vector.bn_aggr(out=agg[:], in_=bst[layer, m][:])
                # sum = mean*npts ; sumsq = (var + mean^2)*npts
                nc.vector.tensor_scalar_mul(stat[:, 0:1], agg[:, 0:1], npts)
                m2 = small.tile([128, 1], F32, tag=f"m2{layer}{m}")
                nc.vector.tensor_tensor(
                    out=m2[:], in0=agg[:, 0:1], in1=agg[:, 0:1], op=ALU.mult,
                )
                nc.vector.tensor_tensor(
                    out=m2[:], in0=agg[:, 1:2], in1=m2[:], op=ALU.add,
                )
                nc.vector.tensor_scalar_mul(stat[:, 1:2], m2[:], npts)
                # keep these tiny DMAs off the sync engine's HWDGE ring: a
                # long-waiting stats DMA there head-of-line blocks the AR
                # result fetch (and with it the whole downstream pass)
                in_b = dram.tile([128, 2], F32, tag=f"arin{layer}{m}")
                out_b = dram.tile([128, 2], F32, tag=f"arout{layer}{m}")
                nc.scalar.dma_start(out=in_b[:], in_=stat[:])
                nc.gpsimd.collective_compute(
                    "AllReduce",
                    ALU.add,
                    replica_groups=[list(range(N_CORES))],
                    ins=[in_b.opt()],
                    outs=[out_b.opt()],
                )
                red = small.tile([128, 2], F32, tag=f"red{layer}{m}")
                nc.gpsimd.dma_start(out=red[:], in_=out_b[:])
                return red

            def bn_coeffs(red, layer, m):
                """scale = gamma*rsqrt(var+eps); bias = beta - mean*scale."""
                name = f"{layer}{m}"
                mean = small.tile([128, 1], F32, tag=f"mean{name}")
                nc.vector.tensor_scalar_mul(mean[:], red[:, 0:1], 1.0 / NTOT)
                ex2 = small.tile([128, 1], F32, tag=f"ex2{name}")
                nc.vector.tensor_scalar_mul(ex2[:], red[:, 1:2], 1.0 / NTOT)
                var = small.tile([128, 1], F32, tag=f"var{name}")
                nc.vector.tensor_tensor(
                    out=var[:], in0=mean[:], in1=mean[:], op=ALU.mult
                )
                nc.vector.tensor_tensor(
                    out=var[:], in0=ex2[:], in1=var[:], op=ALU.subtract
                )
                std = small.tile([128, 1], F32, tag=f"std{name}")
                nc.scalar.activation(
                    out=std[:], in_=var[:], func=AF.Sqrt, bias=eps_sb[:]
                )
                inv = small.tile([128, 1], F32, tag=f"inv{name}")
                nc.vector.reciprocal(out=inv[:], in_=std[:])
                gcol = 4 * m if layer == 1 else 4 * m + 2
                bcol = gcol + 1
                sc = small.tile([128, 1], F32, tag=f"sc{name}")
                nc.vector.tensor_tensor(
                    out=sc[:], in0=inv[:], in1=bnp[:, gcol: gcol + 1],
                    op=ALU.mult,
                )
                bi = small.tile([128, 1], F32, tag=f"bi{name}")
                nc.vector.tensor_tensor(
                    out=bi[:], in0=mean[:], in1=sc[:], op=ALU.mult
                )
                nc.vector.tensor_tensor(
                    out=bi[:], in0=bnp[:, bcol: bcol + 1], in1=bi[:],
                    op=ALU.subtract,
                )
                return sc, bi

            def sign_chunks(m, a, q, sc, bi):
                """xs2 plane m interior <- sign(v[m]*sc + bi), images a..a+q."""
                nc.scalar.activation(
                    out=_interior(xs2_all, m, a, q),
                    in_=v[m][:, a * 784: (a + q) * 784].rearrange(
                        "p (n r c) -> p n r c", r=28, c=28),
                    func=AF.Sign,
                    bias=bi[:],
                    scale=sc[:],
                )

            def final_chunk(m, k, sc, bi, dve_affine=False):
                """y chunk <- clip(v[m]*sc + bi, -1, 1) (v holds t2), bf16."""
                a = k * QI
                u = upool.tile([128, QI * 784], BF16, tag="u")
                if dve_affine:
                    nc.vector.tensor_scalar(
                        out=u[:],
                        in0=v[m][:, a * 784: (a + QI) * 784],
                        scalar1=sc[:], scalar2=bi[:],
                        op0=ALU.mult, op1=ALU.add,
                    )
                else:
                    nc.scalar.activation(
                        out=u[:],
                        in_=v[m][:, a * 784: (a + QI) * 784],
                        func=AF.Identity, bias=bi[:], scale=sc[:],
                    )
                nc.vector.tensor_scalar(
                    out=u[:], in0=u[:],
                    scalar1=-1.0, scalar2=1.0,
                    op0=ALU.max, op1=ALU.min,
                )
                nc.sync.dma_start(
                    out=_x_dram_ap(yd, a, QI, m),
                    in_=u.rearrange("p (n f) -> p n f", n=QI),
                )

            # ================= conv1 m=0 =================
            conv_half(1, xs_all, 0)
            red10 = start_allreduce(1, 0)
            coef1 = {}

            # ================= conv1 m=1 =================
            # the full m=0 sign pass (conv2 input plane 0) streams in 2-image
            # chunks once red10 lands (~1/4 into this half); ScalarE has no
            # other work here so a late AR only stalls ScalarE, never the PE
            def inj_c1m1(g):
                if g == 4:
                    coef1[0] = bn_coeffs(red10, 1, 0)
                if 4 <= g <= 11:
                    sign_chunks(0, 2 * (g - 4), 2, *coef1[0])

            conv_half(1, xs_all, 1, inject=inj_c1m1)
            red11 = start_allreduce(1, 1, after=red10)
            coef1[1] = bn_coeffs(red11, 1, 1)
            # prime one 2-image chunk of plane 1 so conv2 can start; the rest
            # streams in during conv2 m=0 with 2-group lookahead.
            sign_chunks(1, 0, QI, *coef1[1])

            # ================= conv2 m=0 =================
            def inj_c2m0(g):
                if g <= 6:
                    sign_chunks(1, QI + 2 * g, 2, *coef1[1])

            conv_half(2, xs2_all, 0, inject=inj_c2m0)
            red20 = start_allreduce(2, 0, after=red11)

            # ================= conv2 m=1 =================
            # hide the m=0 output pass (affine+clip+store) under these MMs
            coef2 = {}

            def inj_c2m1(g):
                if g == 2:
                    coef2[0] = bn_coeffs(red20, 2, 0)
                if 2 <= g <= 9:
                    final_chunk(0, g - 2, *coef2[0],
                                dve_affine=((g - 2) % 2 == 1))

            conv_half(2, xs2_all, 1, inject=inj_c2m1)
            red21 = start_allreduce(2, 1, after=red20)
            sc21, bi21 = bn_coeffs(red21, 2, 1)
            # alternate the affine between ScalarE and DVE so the exposed
            # tail pass runs on two lanes
            for k in range(NPC // QI):
                final_chunk(1, k, sc21, bi21, dve_affine=(k % 2 == 1))

    nc.compile()
    # pass-ordering bug in this bacc vintage: late compile passes can leave
    # >1 sync wait on an instruction (HW cap); one more split pass fixes it
    nc.generate_event_semaphores()
    return nc


_NC_CACHE = None
_RUNNER = None


def _get_nc():
    global _NC_CACHE
    if _NC_CACHE is None:
        _NC_CACHE = _build()
    return _NC_CACHE


def _make_runner(nc):
    """Persistent jitted shard_map over 8 cores (mirrors
    bass2jax.run_bass_via_pjrt but cached, so repeat calls skip retracing)."""
    import jax
    import jax.core
    from jax.sharding import Mesh, PartitionSpec
    from jax.experimental.shard_map import shard_map
    from concourse import bass2jax, mybir as mb

    bass2jax.install_neuronx_cc_hook()
    partition_name = (
        nc.partition_id_tensor.name if nc.partition_id_tensor else None
    )
    in_names, out_names, out_avals, zero_outs = [], [], [], []
    for alloc in nc.m.functions[0].allocations:
        if not isinstance(alloc, mb.MemoryLocationSet):
            continue
        name = alloc.memorylocations[0].name
        if alloc.kind == "ExternalInput":
            if name != partition_name:
                in_names.append(name)
        elif alloc.kind == "ExternalOutput":
            shape = tuple(alloc.tensor_shape)
            dtype = mb.dt.np(alloc.dtype)
            out_names.append(name)
            out_avals.append(jax.core.ShapedArray(shape, dtype))
            zero_outs.append(np.zeros(shape, dtype))
    n_params = len(in_names)
    n_outs = len(out_avals)
    all_in_names = list(in_names) + list(out_names)
    if partition_name is not None:
        all_in_names.append(partition_name)
    donate = tuple(range(n_params, n_params + n_outs))

    def _body(*args):
        operands = list(args)
        if partition_name is not None:
            operands.append(bass2jax.partition_id_tensor())
        outs = bass2jax._bass_exec_p.bind(
            *operands,
            out_avals=tuple(out_avals),
            in_names=tuple(all_in_names),
            out_names=tuple(out_names),
            lowering_input_output_aliases=(),
            sim_require_finite=True,
            sim_require_nnan=True,
            nc=nc,
        )
        return tuple(outs)

    devices = jax.devices()[:N_CORES]
    mesh = Mesh(np.asarray(devices), ("core",))
    in_specs = (PartitionSpec("core"),) * (n_params + n_outs)
    out_specs = (PartitionSpec("core"),) * n_outs
    sharded = jax.jit(
        shard_map(
            _body, mesh=mesh, in_specs=in_specs, out_specs=out_specs,
            check_rep=False,
        ),
        donate_argnums=donate,
        keep_unused=True,
    )
    return sharded, in_names, out_names, zero_outs


def _get_runner():
    global _RUNNER
    if _RUNNER is None:
        _RUNNER = _make_runner(_get_nc())
    return _RUNNER


def _prep_x(x):
    """x -> bf16 (halves the input DMA bytes; the +-0.4% residual rounding
    is far inside the 2e-2 gate)."""
    return np.asarray(x, dtype=np.float32).astype(ml_dtypes.bfloat16)


def _prep_weights(w):
    """[O=256,I=256,3,3] f32 -> sign-binarized DoubleRow lhsT layout
    [ki=128, off=9, ko=2, o=256] fp8: [ki,off,ko,o] = sign(w[o, ko*128+ki, off])."""
    ws = np.sign(np.asarray(w, dtype=np.float32))
    # [o, ko, ki, off] -> [ki, off, ko, o]
    ws = ws.reshape(256, 2, 128, 9).transpose(2, 3, 1, 0)
    return np.ascontiguousarray(ws.astype(NP_FP8))


def kernel(x, w1, b1, g1, be1, w2, b2, g2, be2):
    x = _prep_x(x)
    w1s = _prep_weights(w1)
    w2s = _prep_weights(w2)
    # per-partition BN params: [128, 8] cols = (g1,be1,g2,be2) for m=0, then m=1
    bnp = np.stack(
        [
            np.asarray(g1, np.float32).reshape(2, 128),
            np.asarray(be1, np.float32).reshape(2, 128),
            np.asarray(g2, np.float32).reshape(2, 128),
            np.asarray(be2, np.float32).reshape(2, 128),
        ],
        axis=-1,
    )  # [2, 128, 4]
    bnp = np.ascontiguousarray(bnp.transpose(1, 0, 2).reshape(128, 8))

    sharded, in_names, out_names, zero_outs = _get_runner()
    per_core = {
        "x": x.reshape(N_CORES * NPC, C, H, W),
        "w1s": np.concatenate([w1s] * N_CORES, axis=0),
        "w2s": np.concatenate([w2s] * N_CORES, axis=0),
        "bnp": np.concatenate([bnp] * N_CORES, axis=0),
    }
    concat_in = [per_core[name] for name in in_names]
    concat_zeros = [
        np.zeros((N_CORES * z.shape[0], *z.shape[1:]), z.dtype)
        for z in zero_outs
    ]
    out_arrs = sharded(*concat_in, *concat_zeros)
    yi = out_names.index("y")
    return np.asarray(out_arrs[yi]).astype(np.float32).reshape(N_FULL, C, H, W)
